# revision 1
# baseline (speedup 1.0000x reference)
"""GQA kernel for Trainium2, sharded over 8 NeuronCores.

Sharding: tensor-parallel over heads. Core g owns Q heads 4g..4g+3 and KV
group g (GQA rep=4, so all 4 local heads share one K/V). The reference's
final projection contracts over the *sequence* axis (faithful swapaxes
quirk), so output rows partition cleanly by head: core g produces rows
g*256..(g+1)*256 of the [2, 2048, 2048] output. No collectives.

Device dataflow per core, per batch b:
  XT = X[b].T (host-prepped, bf16)    [hidden, seq]
  QT = wq.T @ XT   (PE, psum accum over hidden chunks)   [256, 2048]
  KVT = wkv.T @ XT                                        [128, 2048]
  V  = transpose(KVT[64:128]) via PE identity-matmul, plus ones column
  per head h: scoresT[k,q] = KT.T @ QT_h  (K=64 contraction)
              probsT = exp(scoresT)  (ACT, scale folded into wq on host)
              avT[65, q] = V'.T @ probsT  (accum over k chunks; row 64 =
                                           softmax denominator)
              attn[q, d] = transpose(avT) * (1/denominator)  per 128-q block
  out rows = attn_nat.T @ wo  (contract over q/seq), + b_o
"""
import numpy as np
import ml_dtypes

import concourse.bass as bass
import concourse.bacc as bacc
import concourse.mybir as mybir
import concourse.tile as tile
from concourse import bass_utils
from concourse.masks import make_identity

BF16 = mybir.dt.bfloat16
F32 = mybir.dt.float32
NP_BF16 = ml_dtypes.bfloat16

B, S, HID = 2, 2048, 2048
NCORES = 8
HEADS_PER_CORE = 4   # of 32
D = 64               # head dim
QF = HEADS_PER_CORE * D   # 256 q-features per core
P = 128
HC = HID // P        # 16 hidden chunks
SC = S // P          # 16 seq chunks

_CACHE = {}


def _build():
    nc = bacc.Bacc("TRN2", target_bir_lowering=False, debug=False,
                   num_devices=NCORES)
    # ---- DRAM I/O ----
    xt_d = nc.dram_tensor("xt", [B, HID, S], BF16, kind="ExternalInput").ap()
    wq_d = nc.dram_tensor("wq", [HID, QF], BF16, kind="ExternalInput").ap()
    bq_d = nc.dram_tensor("bq", [2, P], F32, kind="ExternalInput").ap()
    wkv_d = nc.dram_tensor("wkv", [HID, P], BF16, kind="ExternalInput").ap()
    bkv_d = nc.dram_tensor("bkv", [P, 1], F32, kind="ExternalInput").ap()
    wo_d = nc.dram_tensor("wo", [HID, HID], BF16, kind="ExternalInput").ap()
    bo_d = nc.dram_tensor("bo", [P, HID], F32, kind="ExternalInput").ap()
    out_d = nc.dram_tensor("out", [B, QF, HID], F32, kind="ExternalOutput").ap()

    with tile.TileContext(nc) as tc:
        with (
            tc.tile_pool(name="consts", bufs=1) as consts,
            tc.tile_pool(name="xt", bufs=1) as xt_pool,
            tc.tile_pool(name="qt", bufs=2) as qt_pool,
            tc.tile_pool(name="kvt", bufs=1) as kvt_pool,
            tc.tile_pool(name="vp", bufs=2) as vp_pool,
            tc.tile_pool(name="pt", bufs=2) as pt_pool,
            tc.tile_pool(name="attnT", bufs=1) as attnT_pool,
            tc.tile_pool(name="attn", bufs=2) as attn_pool,
            tc.tile_pool(name="wos", bufs=1) as wos_pool,
            tc.tile_pool(name="outp", bufs=2) as out_pool,
            tc.tile_pool(name="rcp", bufs=4) as rcp_pool,
            tc.tile_pool(name="psum", bufs=1, space="PSUM") as psum,
        ):
            # ---- constants ----
            wq_sb = consts.tile([P, HC, QF], BF16)
            nc.sync.dma_start(wq_sb, wq_d.rearrange("(hc p) q -> p hc q", p=P))
            wkv_sb = consts.tile([P, HC, P], BF16)
            nc.sync.dma_start(wkv_sb, wkv_d.rearrange("(hc p) q -> p hc q", p=P))
            bq_sb = consts.tile([P, 2], F32)
            nc.sync.dma_start(bq_sb, bq_d.rearrange("c p -> p c"))
            bkv_sb = consts.tile([P, 1], F32)
            nc.sync.dma_start(bkv_sb, bkv_d)
            bo_sb = consts.tile([P, HID], F32)
            nc.sync.dma_start(bo_sb, bo_d)
            ident = consts.tile([P, P], BF16)
            make_identity(nc, ident)
            # shifted identity block at partitions 64-127, cols 0-63 (so the
            # V-transpose operands share base_partition 64)
            ident2 = consts.tile([P, P], BF16)
            nc.sync.dma_start(ident2[64:128, 0:64], ident[0:64, 0:64])

            attn_nat = {}   # per-batch normalized attention, [q, (h d)] bf16

            for b in range(B):
                # ---- load XT[b] ----
                xt_sb = xt_pool.tile([P, HC, S], BF16, tag="xt")
                for hc in range(HC):
                    nc.sync.dma_start(xt_sb[:, hc, :],
                                      xt_d[b, hc * P:(hc + 1) * P, :])

                # ---- QT projection: [256, 2048] ----
                qt_sb = qt_pool.tile([P, 2, S], BF16, tag="qt")
                for qc in range(2):
                    for sh in range(2):
                        ps = psum.tile([P, 1024], F32, tag="sc", bufs=2)
                        for j in range(2):
                            for hc in range(HC):
                                nc.tensor.matmul(
                                    ps[:, j * 512:(j + 1) * 512],
                                    lhsT=wq_sb[:, hc, qc * P:(qc + 1) * P],
                                    rhs=xt_sb[:, hc, sh * 1024 + j * 512:
                                              sh * 1024 + (j + 1) * 512],
                                    start=(hc == 0), stop=(hc == HC - 1))
                        nc.vector.tensor_tensor(
                            out=qt_sb[:, qc, sh * 1024:(sh + 1) * 1024],
                            in0=ps, in1=bq_sb[:, qc:qc + 1].to_broadcast((P, 1024)),
                            op=mybir.AluOpType.add)

                # ---- KVT projection: [128, 2048] (K rows 0-63, V rows 64-127)
                kvt_sb = kvt_pool.tile([P, S], BF16, tag="kvt")
                for sh in range(2):
                    ps = psum.tile([P, 1024], F32, tag="sc", bufs=2)
                    for j in range(2):
                        for hc in range(HC):
                            nc.tensor.matmul(
                                ps[:, j * 512:(j + 1) * 512],
                                lhsT=wkv_sb[:, hc, :],
                                rhs=xt_sb[:, hc, sh * 1024 + j * 512:
                                          sh * 1024 + (j + 1) * 512],
                                start=(hc == 0), stop=(hc == HC - 1))
                    nc.vector.tensor_tensor(
                        out=kvt_sb[:, sh * 1024:(sh + 1) * 1024],
                        in0=ps, in1=bkv_sb[:, 0:1].to_broadcast((P, 1024)),
                        op=mybir.AluOpType.add)

                # replicate KT into partitions 64-127 so odd heads' scores
                # matmuls have matching operand base partitions
                kt2_sb = kvt_pool.tile([P, S], BF16, tag="kt2")
                nc.sync.dma_start(kt2_sb[64:128, :], kvt_sb[0:64, :])

                # ---- V natural + ones column: [k, 65] per k-chunk ----
                vp_sb = vp_pool.tile([P, SC, 65], BF16, tag="vp")
                nc.vector.memset(vp_sb[:, :, 64], 1.0)
                for kc in range(SC):
                    tr = psum.tile([P, 64], BF16, tag="tr", bufs=2)
                    nc.tensor.transpose(
                        tr, kvt_sb[64:128, kc * P:(kc + 1) * P],
                        ident2[64:128, 0:64])
                    nc.vector.tensor_copy(out=vp_sb[:, kc, 0:64], in_=tr)

                # ---- attention per local head ----
                attn_sb = attn_pool.tile([P, SC, QF], BF16, tag="attn")
                attn_nat[b] = attn_sb
                for h in range(4):
                    pbase = (h % 2) * 64
                    qt_h = qt_sb[pbase:pbase + 64, h // 2, :]   # [64, 2048]
                    kt_h = (kvt_sb if h % 2 == 0 else kt2_sb)[pbase:pbase + 64, :]
                    attnT_sb = attnT_pool.tile([65, S], BF16, tag="attnT")
                    for qtp in range(2):
                        av = psum.tile([P, 1024], F32, tag="av", bufs=1)
                        for kc in range(SC):
                            sc_ps = psum.tile([P, 1024], F32, tag="sc", bufs=2)
                            for j in range(2):
                                nc.tensor.matmul(
                                    sc_ps[:, j * 512:(j + 1) * 512],
                                    lhsT=kt_h[:, kc * P:(kc + 1) * P],
                                    rhs=qt_h[:, qtp * 1024 + j * 512:
                                             qtp * 1024 + (j + 1) * 512],
                                    start=True, stop=True)
                            pt = pt_pool.tile([P, 1024], BF16, tag="pt")
                            nc.scalar.activation(
                                pt, sc_ps, mybir.ActivationFunctionType.Exp)
                            for j in range(2):
                                nc.tensor.matmul(
                                    av[0:65, j * 512:(j + 1) * 512],
                                    lhsT=vp_sb[:, kc, :],
                                    rhs=pt[:, j * 512:(j + 1) * 512],
                                    start=(kc == 0), stop=(kc == SC - 1),
                                    skip_group_check=True)
                        nc.vector.tensor_copy(
                            out=attnT_sb[:, qtp * 1024:(qtp + 1) * 1024],
                            in_=av[0:65, :])
                    # transpose + normalize into attn_nat[:, :, h*64:(h+1)*64]
                    for tb in range(SC):
                        tr2 = psum.tile([P, 65], BF16, tag="tr", bufs=2)
                        nc.tensor.transpose(
                            tr2, attnT_sb[:, tb * P:(tb + 1) * P],
                            ident[0:65, 0:65])
                        rcp = rcp_pool.tile([P, 1], F32, tag="rcp")
                        nc.vector.reciprocal(rcp, tr2[:, 64:65])
                        nc.vector.tensor_tensor(
                            out=attn_sb[:, tb, h * D:(h + 1) * D],
                            in0=tr2[:, 0:64],
                            in1=rcp.to_broadcast((P, 64)),
                            op=mybir.AluOpType.mult)

            # ---- final projection: out[b, r, :] = attn_nat.T @ wo + bo ----
            for jh in range(2):
                wo_sl = wos_pool.tile([P, SC, 1024], BF16, tag="wo")
                for sq in range(SC):
                    nc.sync.dma_start(
                        wo_sl[:, sq, :],
                        wo_d[sq * P:(sq + 1) * P, jh * 1024:(jh + 1) * 1024])
                for b in range(B):
                    for rc in range(2):
                        ps = psum.tile([P, 1024], F32, tag="sc", bufs=2)
                        for j in range(2):
                            for sq in range(SC):
                                nc.tensor.matmul(
                                    ps[:, j * 512:(j + 1) * 512],
                                    lhsT=attn_nat[b][:, sq, rc * P:(rc + 1) * P],
                                    rhs=wo_sl[:, sq, j * 512:(j + 1) * 512],
                                    start=(sq == 0), stop=(sq == SC - 1))
                        out_sb = out_pool.tile([P, 1024], F32, tag="out")
                        nc.vector.tensor_tensor(
                            out=out_sb, in0=ps,
                            in1=bo_sb[:, jh * 1024:(jh + 1) * 1024],
                            op=mybir.AluOpType.add)
                        nc.sync.dma_start(
                            out_d[b, rc * P:(rc + 1) * P,
                                  jh * 1024:(jh + 1) * 1024],
                            out_sb)

    nc.compile()
    return nc


def _get_nc():
    if "nc" not in _CACHE:
        _CACHE["nc"] = _build()
    return _CACHE["nc"]


def _prep_inputs(hidden_state, w_q, b_q, w_k, b_k, w_v, b_v, w_o, b_o):
    """Host-side sharding/layout prep. Only layout/dtype transforms."""
    xt = np.ascontiguousarray(hidden_state.transpose(0, 2, 1)).astype(NP_BF16)
    wo = np.ascontiguousarray(w_o).astype(NP_BF16)
    bo = np.broadcast_to(b_o.astype(np.float32), (P, HID)).copy()
    in_maps = []
    for g in range(NCORES):
        wq_g = np.ascontiguousarray(
            w_q[:, g * QF:(g + 1) * QF] * 0.125).astype(NP_BF16)
        bq_g = np.ascontiguousarray(
            (b_q[g * QF:(g + 1) * QF] * 0.125).reshape(2, P)).astype(np.float32)
        wkv_g = np.ascontiguousarray(np.concatenate(
            [w_k[:, g * D:(g + 1) * D], w_v[:, g * D:(g + 1) * D]],
            axis=1)).astype(NP_BF16)
        bkv_g = np.ascontiguousarray(np.concatenate(
            [b_k[g * D:(g + 1) * D], b_v[g * D:(g + 1) * D]])
            .reshape(P, 1)).astype(np.float32)
        in_maps.append({
            "xt": xt, "wq": wq_g, "bq": bq_g, "wkv": wkv_g, "bkv": bkv_g,
            "wo": wo, "bo": bo,
        })
    return in_maps


def kernel(hidden_state, w_q, b_q, w_k, b_k, w_v, b_v, w_o, b_o,
           _trace=False):
    hidden_state = np.asarray(hidden_state, np.float32)
    args = [np.asarray(a, np.float32) for a in
            (w_q, b_q, w_k, b_k, w_v, b_v, w_o, b_o)]
    nc = _get_nc()
    in_maps = _prep_inputs(hidden_state, *args)
    res = bass_utils.run_bass_kernel_spmd(
        nc, in_maps, core_ids=list(range(NCORES)), trace=_trace)
    out = np.concatenate([res.results[g]["out"] for g in range(NCORES)],
                         axis=1).astype(np.float32)
    if _trace:
        _CACHE["last_results"] = res
    return out



# revision 9
# speedup vs baseline: 1.1082x; 1.1082x over previous
"""GQA kernel for Trainium2, sharded over 8 NeuronCores.

Sharding: tensor-parallel over heads. Core g owns Q heads 4g..4g+3 and KV
group g (GQA rep=4, so all 4 local heads share one K/V). The reference's
final projection contracts over the *sequence* axis (faithful swapaxes
quirk), so output rows partition cleanly by head: core g produces rows
g*256..(g+1)*256 of the [2, 2048, 2048] output. No collectives.

v2 schedule (software-pipelined, trace-driven):
  - xt is loaded per-hidden-chunk; projection matmuls start as chunks land.
  - attention inner loop is ACT(exp)-bound; batch-1 projection and batch-0
    out-projection matmul chains are injected between attention matmuls as
    "filler" thunks so the PE never idles while ACT chews exp tiles.
  - all transposes (V, attn, softmax denominator) run on the DMA XBAR
    (dma_start_transpose), not the PE.
  - out-projection is split per head-pair so rows for heads 0/1 of batch 1
    are projected while heads 2/3 still run attention; only the last
    quarter of the out-projection remains after the attention pipeline.
"""
import numpy as np
import ml_dtypes
from collections import deque

import concourse.bass as bass
import concourse.bacc as bacc
import concourse.mybir as mybir
import concourse.tile as tile
from concourse import bass_utils

BF16 = mybir.dt.bfloat16
F32 = mybir.dt.float32
NP_BF16 = ml_dtypes.bfloat16

B, S, HID = 2, 2048, 2048
NCORES = 8
HEADS_PER_CORE = 4   # of 32
D = 64               # head dim
QF = HEADS_PER_CORE * D   # 256 q-features per core
P = 128
HC = HID // P        # 16 hidden chunks
SC = S // P          # 16 seq chunks

_CACHE = {}


def _build():
    nc = bacc.Bacc("TRN2", target_bir_lowering=False, debug=False,
                   num_devices=NCORES)
    # ---- DRAM I/O ----
    xt_d = nc.dram_tensor("xt", [B, HID, S], BF16, kind="ExternalInput").ap()
    wq_d = nc.dram_tensor("wq", [HID, QF], BF16, kind="ExternalInput").ap()
    bq_d = nc.dram_tensor("bq", [2, P], F32, kind="ExternalInput").ap()
    wkv_d = nc.dram_tensor("wkv", [HID, P], BF16, kind="ExternalInput").ap()
    bkv_d = nc.dram_tensor("bkv", [P, 1], F32, kind="ExternalInput").ap()
    wo_d = nc.dram_tensor("wo", [HID, HID], BF16, kind="ExternalInput").ap()
    bo_d = nc.dram_tensor("bo", [P, HID], F32, kind="ExternalInput").ap()
    out_d = nc.dram_tensor("out", [B, QF, HID], F32, kind="ExternalOutput").ap()

    with tile.TileContext(nc) as tc:
        with (
            tc.tile_pool(name="consts", bufs=1) as consts,
            tc.tile_pool(name="xt", bufs=1) as xt_pool,
            tc.tile_pool(name="qt", bufs=2) as qt_pool,
            tc.tile_pool(name="kvt", bufs=2) as kvt_pool,
            tc.tile_pool(name="vp", bufs=2) as vp_pool,
            tc.tile_pool(name="pt", bufs=2) as pt_pool,
            tc.tile_pool(name="attnT", bufs=2) as attnT_pool,
            tc.tile_pool(name="araw", bufs=2) as araw_pool,
            tc.tile_pool(name="dxt", bufs=2) as dxt_pool,
            tc.tile_pool(name="rcp", bufs=2) as rcp_pool,
            tc.tile_pool(name="attn", bufs=2) as attn_pool,
            tc.tile_pool(name="wos", bufs=2) as wos_pool,
            tc.tile_pool(name="outp", bufs=2) as out_pool,
            tc.tile_pool(name="psum", bufs=1, space="PSUM") as psum,
        ):
            # ---- constants ----
            wq_sb = consts.tile([P, HC, QF], BF16)
            nc.sync.dma_start(wq_sb, wq_d.rearrange("(hc p) q -> p hc q", p=P))
            wkv_sb = consts.tile([P, HC, P], BF16)
            nc.sync.dma_start(wkv_sb, wkv_d.rearrange("(hc p) q -> p hc q", p=P))
            bq_sb = consts.tile([P, 2], F32)
            nc.sync.dma_start(bq_sb, bq_d.rearrange("c p -> p c"))
            bkv_sb = consts.tile([P, 1], F32)
            nc.sync.dma_start(bkv_sb, bkv_d)
            bo_sb = consts.tile([P, HID], F32)
            nc.sync.dma_start(bo_sb, bo_d)

            # filler thunk queue: each thunk emits a small chunk of deferred
            # PE work (or a DMA, cost 0) and returns its PE cost in matmuls.
            work = deque()

            def pump(budget=1):
                spent = 0
                while work and spent < budget:
                    spent += work.popleft()()

            def flush():
                while work:
                    work.popleft()()

            # ---------------- projections ----------------
            def load_xt_chunk(b, hc):
                t = xt_pool.tile([P, S], BF16, tag=f"xt{hc}",
                                 name=f"xtc{hc}")
                nc.sync.dma_start(t, xt_d[b, hc * P:(hc + 1) * P, :])
                return t

            def proj_drain_qt(qt_sb, ps, qc, sh):
                nc.vector.tensor_tensor(
                    out=qt_sb[:, qc, sh * 1024:(sh + 1) * 1024],
                    in0=ps, in1=bq_sb[:, qc:qc + 1].to_broadcast((P, 1024)),
                    op=mybir.AluOpType.add)

            def proj_drain_kv(kvt_sb, ps, sh, j):
                nc.vector.tensor_tensor(
                    out=kvt_sb[:, sh * 1024 + j * 512:sh * 1024 + (j + 1) * 512],
                    in0=ps, in1=bkv_sb[:, 0:1].to_broadcast((P, 512)),
                    op=mybir.AluOpType.add)

            def proj_phase_b0(xt_t, qt_sb, kvt_sb):
                """Paced projection for batch 0: matmuls chase the xt DMAs."""
                for rnd, qc in enumerate(range(2)):
                    a0 = psum.tile([P, 1024], F32, tag="A", bufs=2)
                    a1 = psum.tile([P, 1024], F32, tag="A", bufs=2)
                    c0 = psum.tile([P, 512], F32, tag="C", bufs=2)
                    c1 = psum.tile([P, 512], F32, tag="C", bufs=2)
                    sh_kv = rnd
                    for hc in range(HC):
                        st, sp = hc == 0, hc == HC - 1
                        for sh, ps in ((0, a0), (1, a1)):
                            for j in range(2):
                                nc.tensor.matmul(
                                    ps[:, j * 512:(j + 1) * 512],
                                    lhsT=wq_sb[:, hc, qc * P:(qc + 1) * P],
                                    rhs=xt_t[hc][:, sh * 1024 + j * 512:
                                                 sh * 1024 + (j + 1) * 512],
                                    start=st, stop=sp, skip_group_check=True)
                        for j, ps in ((0, c0), (1, c1)):
                            nc.tensor.matmul(
                                ps,
                                lhsT=wkv_sb[:, hc, :],
                                rhs=xt_t[hc][:, sh_kv * 1024 + j * 512:
                                             sh_kv * 1024 + (j + 1) * 512],
                                start=st, stop=sp, skip_group_check=True)
                    proj_drain_qt(qt_sb, a0, qc, 0)
                    proj_drain_qt(qt_sb, a1, qc, 1)
                    proj_drain_kv(kvt_sb, c0, sh_kv, 0)
                    proj_drain_kv(kvt_sb, c1, sh_kv, 1)

            def make_proj_thunks(b, xt_t, qt_sb, kvt_sb):
                """Deferred projections for batch b, emitted as fillers."""
                chains = ([("qt", qc, sh, j) for qc in range(2)
                           for sh in range(2) for j in range(2)]
                          + [("kv", None, sh, j) for sh in range(2)
                             for j in range(2)])

                def chain_thunks(kind, qc, sh, j):
                    ps_box = {}

                    def start_thunk():
                        ps_box["ps"] = psum.tile([P, 512], F32, tag="C", bufs=2,
                                                 name="projps")
                        return 0

                    def mk_mm(t):
                        def mm_thunk():
                            ps = ps_box["ps"]
                            for hc in (2 * t, 2 * t + 1):
                                st, sp = hc == 0, hc == HC - 1
                                if kind == "qt":
                                    nc.tensor.matmul(
                                        ps,
                                        lhsT=wq_sb[:, hc, qc * P:(qc + 1) * P],
                                        rhs=xt_t[hc][:, sh * 1024 + j * 512:
                                                     sh * 1024 + (j + 1) * 512],
                                        start=st, stop=sp,
                                        skip_group_check=True)
                                else:
                                    nc.tensor.matmul(
                                        ps,
                                        lhsT=wkv_sb[:, hc, :],
                                        rhs=xt_t[hc][:, sh * 1024 + j * 512:
                                                     sh * 1024 + (j + 1) * 512],
                                        start=st, stop=sp,
                                        skip_group_check=True)
                            return 2
                        return mm_thunk

                    def drain_thunk():
                        ps = ps_box["ps"]
                        if kind == "qt":
                            nc.vector.tensor_tensor(
                                out=qt_sb[:, qc, sh * 1024 + j * 512:
                                          sh * 1024 + (j + 1) * 512],
                                in0=ps,
                                in1=bq_sb[:, qc:qc + 1].to_broadcast((P, 512)),
                                op=mybir.AluOpType.add)
                        else:
                            proj_drain_kv(kvt_sb, ps, sh, j)
                        return 0

                    yield start_thunk
                    for t in range(HC // 2):
                        yield mk_mm(t)
                    yield drain_thunk

                for ch in chains:
                    for th in chain_thunks(*ch):
                        work.append(th)

            def finish_kv(b, kvt_sb):
                """K replica at partitions 64-127 + V transpose via XBAR."""
                kt2_sb = kvt_pool.tile([P, S], BF16, tag="kt2")
                nc.sync.dma_start(kt2_sb[64:128, :], kvt_sb[0:64, :])
                # XBAR transpose needs a contiguous destination; land V there
                # and splice the softmax-denominator ones column on the DVE.
                vpt = vp_pool.tile([P, SC, D], BF16, tag="vpt")
                nc.sync.dma_start_transpose(vpt, kvt_sb[64:128, :])
                vp_sb = vp_pool.tile([P, SC, 65], BF16, tag="vp")
                nc.vector.memset(vp_sb[:, :, 64], 1.0)
                nc.vector.tensor_copy(out=vp_sb[:, :, 0:64], in_=vpt)
                return kt2_sb, vp_sb

            # ---------------- out-projection chains ----------------
            def make_outproj_thunks(b, parts):
                """Project rows of batch b; parts = [(attn_half, rc), ...].
                One wo column-slice load per jq, shared by all row chains."""
                for jq in range(4):
                    wo_box = {}

                    def mk_load(jq=jq):
                        def load_thunk():
                            w = wos_pool.tile([P, SC, 512], BF16, tag="wo",
                                              name="wosl")
                            nc.sync.dma_start(
                                w, wo_d.rearrange("(sq p) j -> p sq j", p=P)
                                [:, :, jq * 512:(jq + 1) * 512])
                            wo_box["w"] = w
                            return 0
                        return load_thunk

                    def mk_start():
                        def start_thunk():
                            wo_box["ps"] = psum.tile([P, 512], F32, tag="C",
                                                     bufs=2, name="outps")
                            return 0
                        return start_thunk

                    def mk_mm(t, half):
                        def mm_thunk():
                            ps = wo_box["ps"]
                            for sq in (2 * t, 2 * t + 1):
                                nc.tensor.matmul(
                                    ps,
                                    lhsT=half[:, sq, :],
                                    rhs=wo_box["w"][:, sq, :],
                                    start=sq == 0, stop=sq == SC - 1,
                                    skip_group_check=True)
                            return 2
                        return mm_thunk

                    def mk_drain(rc, jq=jq, b=b):
                        def drain_thunk():
                            ps = wo_box["ps"]
                            o = out_pool.tile([P, 512], F32, tag="out")
                            nc.vector.tensor_tensor(
                                out=o, in0=ps,
                                in1=bo_sb[:, jq * 512:(jq + 1) * 512],
                                op=mybir.AluOpType.add)
                            nc.sync.dma_start(
                                out_d[b, rc * P:(rc + 1) * P,
                                      jq * 512:(jq + 1) * 512], o)
                            return 0
                        return drain_thunk

                    work.append(mk_load())
                    for half, rc in parts:
                        work.append(mk_start())
                        for t in range(SC // 2):
                            work.append(mk_mm(t, half))
                        work.append(mk_drain(rc))

            # ---------------- attention ----------------
            def attn_phase(b, qt_sb, kvt_sb, kt2_sb, vp_sb, attn_halves,
                           gate_hook=None):
                pair_state = {}
                for h in range(4):
                    pbase = (h % 2) * 64
                    qt_h = qt_sb[pbase:pbase + 64, h // 2, :]
                    kt_h = (kvt_sb if h % 2 == 0 else kt2_sb)[
                        pbase:pbase + 64, :]
                    attnT_sb = attnT_pool.tile([65, S], BF16, tag="attnT")
                    for qtp in range(2):
                        av = psum.tile([P, 1024], F32, tag="av", bufs=1)
                        prev = None
                        for kc in range(SC):
                            sc_ps = psum.tile([P, 1024], F32, tag="A", bufs=2)
                            for j in range(2):
                                nc.tensor.matmul(
                                    sc_ps[:, j * 512:(j + 1) * 512],
                                    lhsT=kt_h[:, kc * P:(kc + 1) * P],
                                    rhs=qt_h[:, qtp * 1024 + j * 512:
                                             qtp * 1024 + (j + 1) * 512],
                                    start=True, stop=True)
                            if prev is not None:
                                ppt, pkc = prev
                                for j in range(2):
                                    nc.tensor.matmul(
                                        av[0:65, j * 512:(j + 1) * 512],
                                        lhsT=vp_sb[:, pkc, :],
                                        rhs=ppt[:, j * 512:(j + 1) * 512],
                                        start=(pkc == 0), stop=(pkc == SC - 1),
                                        skip_group_check=True)
                            if kc % 2 == 1:
                                pump(1)
                            pt = pt_pool.tile([P, 1024], BF16, tag="pt")
                            nc.scalar.activation(
                                pt, sc_ps, mybir.ActivationFunctionType.Exp)
                            prev = (pt, kc)
                        ppt, pkc = prev
                        for j in range(2):
                            nc.tensor.matmul(
                                av[0:65, j * 512:(j + 1) * 512],
                                lhsT=vp_sb[:, pkc, :],
                                rhs=ppt[:, j * 512:(j + 1) * 512],
                                start=(pkc == 0), stop=(pkc == SC - 1),
                                skip_group_check=True)
                        nc.vector.tensor_copy(
                            out=attnT_sb[:, qtp * 1024:(qtp + 1) * 1024],
                            in_=av[0:65, :])
                        pump(2)
                    # head tail: attn transpose via XBAR, off the PE; the
                    # softmax denominator row (64) is collected per head-pair
                    # into a 16-partition-aligned tile (XBAR offset rule).
                    araw = araw_pool.tile([P, SC, D], BF16,
                                          tag=f"araw{h % 2}")
                    nc.sync.dma_start_transpose(araw, attnT_sb[0:64, :])
                    if h % 2 == 0:
                        den = dxt_pool.tile([16, S], BF16, tag="den")
                        nc.vector.memset(den, 1.0)
                        pair_state.clear()
                        pair_state.update(den=den, araw0=araw)
                    else:
                        den = pair_state["den"]
                        pair_state["araw1"] = araw
                    nc.sync.dma_start(den[h % 2:h % 2 + 1, :],
                                      attnT_sb[64:65, :])
                    if h % 2 == 1:
                        dxt = dxt_pool.tile([P, SC, 16], BF16, tag="dxt")
                        nc.sync.dma_start_transpose(dxt, den)
                        half = attn_halves[h // 2]
                        for hh in range(2):
                            rcp = rcp_pool.tile([P, SC], F32, tag="rcp")
                            nc.vector.reciprocal(rcp, dxt[:, :, hh])
                            ar = pair_state[f"araw{hh}"]
                            eng = nc.vector if hh == 0 else nc.gpsimd
                            for tb in range(SC):
                                eng.tensor_tensor(
                                    out=half[:, tb, hh * D:(hh + 1) * D],
                                    in0=ar[:, tb, :],
                                    in1=rcp[:, tb:tb + 1].to_broadcast((P, D)),
                                    op=mybir.AluOpType.mult)
                    if gate_hook is not None:
                        gate_hook(h)

            # ================= schedule =================
            # batch 0 projections, paced against the xt chunk DMAs
            xt0 = [load_xt_chunk(0, hc) for hc in range(HC)]
            qt0 = qt_pool.tile([P, 2, S], BF16, tag="qt")
            kvt0 = kvt_pool.tile([P, S], BF16, tag="kvt")
            proj_phase_b0(xt0, qt0, kvt0)
            kt20, vp0 = finish_kv(0, kvt0)

            # batch 1 xt load starts as soon as batch 0 mms release chunks
            xt1 = [load_xt_chunk(1, hc) for hc in range(HC)]
            qt1 = qt_pool.tile([P, 2, S], BF16, tag="qt")
            kvt1 = kvt_pool.tile([P, S], BF16, tag="kvt")
            make_proj_thunks(1, xt1, qt1, kvt1)

            attn0 = [attn_pool.tile([P, SC, P], BF16, tag=f"attn{i}",
                                     name=f"attn0_{i}") for i in range(2)]
            attn1 = [attn_pool.tile([P, SC, P], BF16, tag=f"attn{i}",
                                     name=f"attn1_{i}") for i in range(2)]

            # batch 0 attention, consuming batch 1 projection fillers
            attn_phase(0, qt0, kvt0, kt20, vp0, attn0)
            flush()
            kt21, vp1 = finish_kv(1, kvt1)

            # batch 1 attention, consuming out-projection fillers:
            # batch 0 rows first; batch-1 heads 0/1 rows as soon as ready
            make_outproj_thunks(0, [(attn0[0], 0), (attn0[1], 1)])

            def gate(h):
                if h == 1:
                    make_outproj_thunks(1, [(attn1[0], 0)])

            attn_phase(1, qt1, kvt1, kt21, vp1, attn1, gate_hook=gate)
            make_outproj_thunks(1, [(attn1[1], 1)])
            flush()

    nc.compile()
    return nc


def _get_nc():
    if "nc" not in _CACHE:
        _CACHE["nc"] = _build()
    return _CACHE["nc"]


def _prep_inputs(hidden_state, w_q, b_q, w_k, b_k, w_v, b_v, w_o, b_o):
    """Host-side sharding/layout prep. Only layout/dtype transforms."""
    xt = np.ascontiguousarray(hidden_state.transpose(0, 2, 1)).astype(NP_BF16)
    wo = np.ascontiguousarray(w_o).astype(NP_BF16)
    bo = np.broadcast_to(b_o.astype(np.float32), (P, HID)).copy()
    in_maps = []
    for g in range(NCORES):
        wq_g = np.ascontiguousarray(
            w_q[:, g * QF:(g + 1) * QF] * 0.125).astype(NP_BF16)
        bq_g = np.ascontiguousarray(
            (b_q[g * QF:(g + 1) * QF] * 0.125).reshape(2, P)).astype(np.float32)
        wkv_g = np.ascontiguousarray(np.concatenate(
            [w_k[:, g * D:(g + 1) * D], w_v[:, g * D:(g + 1) * D]],
            axis=1)).astype(NP_BF16)
        bkv_g = np.ascontiguousarray(np.concatenate(
            [b_k[g * D:(g + 1) * D], b_v[g * D:(g + 1) * D]])
            .reshape(P, 1)).astype(np.float32)
        in_maps.append({
            "xt": xt, "wq": wq_g, "bq": bq_g, "wkv": wkv_g, "bkv": bkv_g,
            "wo": wo, "bo": bo,
        })
    return in_maps


def kernel(hidden_state, w_q, b_q, w_k, b_k, w_v, b_v, w_o, b_o,
           _trace=False):
    hidden_state = np.asarray(hidden_state, np.float32)
    args = [np.asarray(a, np.float32) for a in
            (w_q, b_q, w_k, b_k, w_v, b_v, w_o, b_o)]
    nc = _get_nc()
    in_maps = _prep_inputs(hidden_state, *args)
    res = bass_utils.run_bass_kernel_spmd(
        nc, in_maps, core_ids=list(range(NCORES)), trace=_trace)
    out = np.concatenate([res.results[g]["out"] for g in range(NCORES)],
                         axis=1).astype(np.float32)
    if _trace:
        _CACHE["last_results"] = res
    return out


# revision 15
# speedup vs baseline: 1.1372x; 1.0262x over previous
"""GQA kernel for Trainium2, sharded over 8 NeuronCores.

Sharding: tensor-parallel over heads. Core g owns Q heads 4g..4g+3 and KV
group g (GQA rep=4, so all 4 local heads share one K/V). The reference's
final projection contracts over the *sequence* axis (faithful swapaxes
quirk), so output rows partition cleanly by head: core g produces rows
g*256..(g+1)*256 of the [2, 2048, 2048] output. No collectives.

v2 schedule (software-pipelined, trace-driven):
  - xt is loaded per-hidden-chunk; projection matmuls start as chunks land.
  - attention inner loop is ACT(exp)-bound; batch-1 projection and batch-0
    out-projection matmul chains are injected between attention matmuls as
    "filler" thunks so the PE never idles while ACT chews exp tiles.
  - all transposes (V, attn, softmax denominator) run on the DMA XBAR
    (dma_start_transpose), not the PE.
  - out-projection is split per head-pair so rows for heads 0/1 of batch 1
    are projected while heads 2/3 still run attention; only the last
    quarter of the out-projection remains after the attention pipeline.
"""
import numpy as np
import ml_dtypes
from collections import deque

import concourse.bass as bass
import concourse.bacc as bacc
import concourse.mybir as mybir
import concourse.tile as tile
from concourse import bass_utils

BF16 = mybir.dt.bfloat16
F32 = mybir.dt.float32
NP_BF16 = ml_dtypes.bfloat16

B, S, HID = 2, 2048, 2048
NCORES = 8
HEADS_PER_CORE = 4   # of 32
D = 64               # head dim
QF = HEADS_PER_CORE * D   # 256 q-features per core
P = 128
HC = HID // P        # 16 hidden chunks
SC = S // P          # 16 seq chunks

_CACHE = {}


def _build():
    nc = bacc.Bacc("TRN2", target_bir_lowering=False, debug=False,
                   num_devices=NCORES)
    # ---- DRAM I/O ----
    xt_d = nc.dram_tensor("xt", [B, HID, S], BF16, kind="ExternalInput").ap()
    # weights arrive pre-arranged on host so every DMA line is contiguous
    wq_d = nc.dram_tensor("wq", [P, HC, QF], BF16, kind="ExternalInput").ap()
    bq_d = nc.dram_tensor("bq", [2, P], F32, kind="ExternalInput").ap()
    wkv_d = nc.dram_tensor("wkv", [P, HC, P], BF16, kind="ExternalInput").ap()
    bkv_d = nc.dram_tensor("bkv", [P, 1], F32, kind="ExternalInput").ap()
    wo_d = nc.dram_tensor("wo", [P, 4, SC, 512], BF16,
                          kind="ExternalInput").ap()
    bo_d = nc.dram_tensor("bo", [P, HID], F32, kind="ExternalInput").ap()
    out_d = nc.dram_tensor("out", [B, QF, HID], F32, kind="ExternalOutput").ap()

    with tile.TileContext(nc) as tc:
        with (
            tc.tile_pool(name="consts", bufs=1) as consts,
            tc.tile_pool(name="xt", bufs=1) as xt_pool,
            tc.tile_pool(name="qt", bufs=2) as qt_pool,
            tc.tile_pool(name="kvt", bufs=2) as kvt_pool,
            tc.tile_pool(name="vp", bufs=2) as vp_pool,
            tc.tile_pool(name="pt", bufs=2) as pt_pool,
            tc.tile_pool(name="attnT", bufs=2) as attnT_pool,
            tc.tile_pool(name="araw", bufs=2) as araw_pool,
            tc.tile_pool(name="dxt", bufs=2) as dxt_pool,
            tc.tile_pool(name="rcp", bufs=2) as rcp_pool,
            tc.tile_pool(name="attn", bufs=2) as attn_pool,
            tc.tile_pool(name="wos", bufs=2) as wos_pool,
            tc.tile_pool(name="outp", bufs=2) as out_pool,
            tc.tile_pool(name="psum", bufs=1, space="PSUM") as psum,
        ):
            # ---- constants ----
            wq_sb = consts.tile([P, HC, QF], BF16)
            nc.sync.dma_start(wq_sb, wq_d)
            wkv_sb = consts.tile([P, HC, P], BF16)
            nc.sync.dma_start(wkv_sb, wkv_d)
            bq_sb = consts.tile([P, 2], F32)
            nc.sync.dma_start(bq_sb, bq_d.rearrange("c p -> p c"))
            bkv_sb = consts.tile([P, 1], F32)
            nc.sync.dma_start(bkv_sb, bkv_d)
            bo_sb = consts.tile([P, HID], F32)
            nc.sync.dma_start(bo_sb, bo_d)

            # filler thunk queue: each thunk emits a small chunk of deferred
            # PE work (or a DMA, cost 0) and returns its PE cost in matmuls.
            work = deque()

            def pump(budget=1):
                spent = 0
                while work and spent < budget:
                    spent += work.popleft()()

            def flush():
                while work:
                    work.popleft()()

            # ---------------- projections ----------------
            def load_xt_chunk(b, hc):
                t = xt_pool.tile([P, S], BF16, tag=f"xt{hc}",
                                 name=f"xtc{hc}")
                nc.sync.dma_start(t, xt_d[b, hc * P:(hc + 1) * P, :])
                return t

            def proj_drain_qt(qt_sb, ps, qc, sh):
                nc.vector.tensor_tensor(
                    out=qt_sb[:, qc, sh * 1024:(sh + 1) * 1024],
                    in0=ps, in1=bq_sb[:, qc:qc + 1].to_broadcast((P, 1024)),
                    op=mybir.AluOpType.add)

            def proj_drain_kv(kvt_sb, ps, sh, j):
                nc.vector.tensor_tensor(
                    out=kvt_sb[:, sh * 1024 + j * 512:sh * 1024 + (j + 1) * 512],
                    in0=ps, in1=bkv_sb[:, 0:1].to_broadcast((P, 512)),
                    op=mybir.AluOpType.add)

            def proj_phase_b0(xt_t, qt_sb, kvt_sb):
                """Paced projection for batch 0: matmuls chase the xt DMAs."""
                for rnd, qc in enumerate(range(2)):
                    a0 = psum.tile([P, 1024], F32, tag="A", bufs=2)
                    a1 = psum.tile([P, 1024], F32, tag="A", bufs=2)
                    c0 = psum.tile([P, 512], F32, tag="C", bufs=2)
                    c1 = psum.tile([P, 512], F32, tag="C", bufs=2)
                    sh_kv = rnd
                    for hc in range(HC):
                        st, sp = hc == 0, hc == HC - 1
                        for sh, ps in ((0, a0), (1, a1)):
                            for j in range(2):
                                nc.tensor.matmul(
                                    ps[:, j * 512:(j + 1) * 512],
                                    lhsT=wq_sb[:, hc, qc * P:(qc + 1) * P],
                                    rhs=xt_t[hc][:, sh * 1024 + j * 512:
                                                 sh * 1024 + (j + 1) * 512],
                                    start=st, stop=sp, skip_group_check=True)
                        for j, ps in ((0, c0), (1, c1)):
                            nc.tensor.matmul(
                                ps,
                                lhsT=wkv_sb[:, hc, :],
                                rhs=xt_t[hc][:, sh_kv * 1024 + j * 512:
                                             sh_kv * 1024 + (j + 1) * 512],
                                start=st, stop=sp, skip_group_check=True)
                    proj_drain_qt(qt_sb, a0, qc, 0)
                    proj_drain_qt(qt_sb, a1, qc, 1)
                    proj_drain_kv(kvt_sb, c0, sh_kv, 0)
                    proj_drain_kv(kvt_sb, c1, sh_kv, 1)

            def make_proj_thunks(b, xt_t, qt_sb, kvt_sb):
                """Deferred projections for batch b, emitted as fillers."""
                chains = ([("qt", qc, sh, j) for qc in range(2)
                           for sh in range(2) for j in range(2)]
                          + [("kv", None, sh, j) for sh in range(2)
                             for j in range(2)])

                def chain_thunks(kind, qc, sh, j):
                    ps_box = {}

                    def start_thunk():
                        ps_box["ps"] = psum.tile([P, 512], F32, tag="C", bufs=2,
                                                 name="projps")
                        return 0

                    def mk_mm(t):
                        def mm_thunk():
                            ps = ps_box["ps"]
                            for hc in (2 * t, 2 * t + 1):
                                st, sp = hc == 0, hc == HC - 1
                                if kind == "qt":
                                    nc.tensor.matmul(
                                        ps,
                                        lhsT=wq_sb[:, hc, qc * P:(qc + 1) * P],
                                        rhs=xt_t[hc][:, sh * 1024 + j * 512:
                                                     sh * 1024 + (j + 1) * 512],
                                        start=st, stop=sp,
                                        skip_group_check=True)
                                else:
                                    nc.tensor.matmul(
                                        ps,
                                        lhsT=wkv_sb[:, hc, :],
                                        rhs=xt_t[hc][:, sh * 1024 + j * 512:
                                                     sh * 1024 + (j + 1) * 512],
                                        start=st, stop=sp,
                                        skip_group_check=True)
                            return 2
                        return mm_thunk

                    def drain_thunk():
                        ps = ps_box["ps"]
                        if kind == "qt":
                            nc.vector.tensor_tensor(
                                out=qt_sb[:, qc, sh * 1024 + j * 512:
                                          sh * 1024 + (j + 1) * 512],
                                in0=ps,
                                in1=bq_sb[:, qc:qc + 1].to_broadcast((P, 512)),
                                op=mybir.AluOpType.add)
                        else:
                            proj_drain_kv(kvt_sb, ps, sh, j)
                        return 0

                    yield start_thunk
                    for t in range(HC // 2):
                        yield mk_mm(t)
                    yield drain_thunk

                for ch in chains:
                    for th in chain_thunks(*ch):
                        work.append(th)

            def finish_kv(b, kvt_sb):
                """K replica at partitions 64-127 + V transpose via XBAR."""
                kt2_sb = kvt_pool.tile([P, S], BF16, tag="kt2")
                nc.sync.dma_start(kt2_sb[64:128, :], kvt_sb[0:64, :])
                # XBAR transpose needs a contiguous destination; land V there
                # and splice the softmax-denominator ones column on the DVE.
                vpt = vp_pool.tile([P, SC, D], BF16, tag="vpt")
                nc.sync.dma_start_transpose(vpt, kvt_sb[64:128, :])
                vp_sb = vp_pool.tile([P, SC, 65], BF16, tag="vp")
                nc.vector.memset(vp_sb[:, :, 64], 1.0)
                nc.vector.tensor_copy(out=vp_sb[:, :, 0:64], in_=vpt)
                return kt2_sb, vp_sb

            # ---------------- out-projection chains ----------------
            wo_slices = {}

            def outproj_load(jq):
                """Thunk: DMA wo column-slice jq into a rotating buffer."""
                def load_thunk():
                    w = wos_pool.tile([P, SC, 512], BF16, tag="wo",
                                      name="wosl")
                    nc.sync.dma_start(w, wo_d[:, jq])
                    wo_slices[jq] = w
                    return 0
                work.append(load_thunk)

            def outproj_chain(b, half, rc, jq):
                """Thunks: one [128,512] row-chunk x col-slice chain."""
                box = {}

                def start_thunk():
                    box["ps"] = psum.tile([P, 512], F32, tag="C",
                                          bufs=2, name="outps")
                    return 0
                work.append(start_thunk)

                def mk_mm(t):
                    def mm_thunk():
                        for sq in (2 * t, 2 * t + 1):
                            nc.tensor.matmul(
                                box["ps"],
                                lhsT=half[:, sq, :],
                                rhs=wo_slices[jq][:, sq, :],
                                start=sq == 0, stop=sq == SC - 1,
                                skip_group_check=True)
                        return 2
                    return mm_thunk
                for t in range(SC // 2):
                    work.append(mk_mm(t))

                def drain_thunk():
                    o = out_pool.tile([P, 512], F32, tag="out")
                    nc.vector.tensor_tensor(
                        out=o, in0=box["ps"],
                        in1=bo_sb[:, jq * 512:(jq + 1) * 512],
                        op=mybir.AluOpType.add)
                    nc.sync.dma_start(
                        out_d[b, rc * P:(rc + 1) * P,
                              jq * 512:(jq + 1) * 512], o)
                    return 0
                work.append(drain_thunk)

            # ---------------- attention ----------------
            def attn_phase(b, qt_sb, kvt_sb, kt2_sb, vp_sb, attn_halves,
                           gate_hook=None):
                pair_state = {}
                for h in range(4):
                    pbase = (h % 2) * 64
                    qt_h = qt_sb[pbase:pbase + 64, h // 2, :]
                    kt_h = (kvt_sb if h % 2 == 0 else kt2_sb)[
                        pbase:pbase + 64, :]
                    attnT_sb = attnT_pool.tile([65, S], BF16, tag="attnT")
                    for qtp in range(2):
                        av = psum.tile([P, 1024], F32, tag="av", bufs=1)
                        prev = None
                        for kc in range(SC):
                            sc_ps = psum.tile([P, 1024], F32, tag="A", bufs=2)
                            for j in range(2):
                                nc.tensor.matmul(
                                    sc_ps[:, j * 512:(j + 1) * 512],
                                    lhsT=kt_h[:, kc * P:(kc + 1) * P],
                                    rhs=qt_h[:, qtp * 1024 + j * 512:
                                             qtp * 1024 + (j + 1) * 512],
                                    start=True, stop=True)
                            if prev is not None:
                                ppt, pkc = prev
                                for j in range(2):
                                    nc.tensor.matmul(
                                        av[0:65, j * 512:(j + 1) * 512],
                                        lhsT=vp_sb[:, pkc, :],
                                        rhs=ppt[:, j * 512:(j + 1) * 512],
                                        start=(pkc == 0), stop=(pkc == SC - 1),
                                        skip_group_check=True)
                            if kc % 2 == 1:
                                pump(1)
                            pt = pt_pool.tile([P, 1024], BF16, tag="pt")
                            nc.scalar.activation(
                                pt, sc_ps, mybir.ActivationFunctionType.Exp)
                            prev = (pt, kc)
                        ppt, pkc = prev
                        for j in range(2):
                            nc.tensor.matmul(
                                av[0:65, j * 512:(j + 1) * 512],
                                lhsT=vp_sb[:, pkc, :],
                                rhs=ppt[:, j * 512:(j + 1) * 512],
                                start=(pkc == 0), stop=(pkc == SC - 1),
                                skip_group_check=True)
                        nc.vector.tensor_copy(
                            out=attnT_sb[:, qtp * 1024:(qtp + 1) * 1024],
                            in_=av[0:65, :])
                        pump(2)
                    # head tail: attn transpose via XBAR, off the PE; the
                    # softmax denominator row (64) is collected per head-pair
                    # into a 16-partition-aligned tile (XBAR offset rule).
                    araw = araw_pool.tile([P, SC, D], BF16,
                                          tag=f"araw{h % 2}")
                    nc.sync.dma_start_transpose(araw, attnT_sb[0:64, :])
                    if h % 2 == 0:
                        den = dxt_pool.tile([16, S], BF16, tag="den")
                        nc.vector.memset(den, 1.0)
                        pair_state.clear()
                        pair_state.update(den=den, araw0=araw)
                    else:
                        den = pair_state["den"]
                        pair_state["araw1"] = araw
                    nc.sync.dma_start(den[h % 2:h % 2 + 1, :],
                                      attnT_sb[64:65, :])
                    if h % 2 == 1:
                        dxt = dxt_pool.tile([P, SC, 16], BF16, tag="dxt")
                        nc.sync.dma_start_transpose(dxt, den)
                        half = attn_halves[h // 2]
                        for hh in range(2):
                            rcp = rcp_pool.tile([P, SC], F32, tag="rcp")
                            nc.vector.reciprocal(rcp, dxt[:, :, hh])
                            ar = pair_state[f"araw{hh}"]
                            eng = nc.vector if hh == 0 else nc.gpsimd
                            for tb in range(SC):
                                eng.tensor_tensor(
                                    out=half[:, tb, hh * D:(hh + 1) * D],
                                    in0=ar[:, tb, :],
                                    in1=rcp[:, tb:tb + 1].to_broadcast((P, D)),
                                    op=mybir.AluOpType.mult)
                    if gate_hook is not None:
                        gate_hook(h)

            # ================= schedule =================
            # batch 0 projections, paced against the xt chunk DMAs
            xt0 = [load_xt_chunk(0, hc) for hc in range(HC)]
            qt0 = qt_pool.tile([P, 2, S], BF16, tag="qt")
            kvt0 = kvt_pool.tile([P, S], BF16, tag="kvt")
            proj_phase_b0(xt0, qt0, kvt0)
            kt20, vp0 = finish_kv(0, kvt0)

            # batch 1 xt load starts as soon as batch 0 mms release chunks
            xt1 = [load_xt_chunk(1, hc) for hc in range(HC)]
            qt1 = qt_pool.tile([P, 2, S], BF16, tag="qt")
            kvt1 = kvt_pool.tile([P, S], BF16, tag="kvt")
            make_proj_thunks(1, xt1, qt1, kvt1)

            attn0 = [attn_pool.tile([P, SC, P], BF16, tag=f"attn{i}",
                                     name=f"attn0_{i}") for i in range(2)]
            attn1 = [attn_pool.tile([P, SC, P], BF16, tag=f"attn{i}",
                                     name=f"attn1_{i}") for i in range(2)]

            # batch 0 attention, consuming batch 1 projection fillers
            attn_phase(0, qt0, kvt0, kt20, vp0, attn0)
            flush()
            kt21, vp1 = finish_kv(1, kvt1)

            # batch 1 attention, consuming out-projection fillers:
            # batch 0 rows first; batch-1 heads 0/1 rows as soon as ready
            for jq in range(4):
                outproj_load(jq)
                outproj_chain(0, attn0[0], 0, jq)
                outproj_chain(0, attn0[1], 1, jq)

            def gate(h):
                if h == 1:
                    for jq in range(2):
                        outproj_load(jq)
                        outproj_chain(1, attn1[0], 0, jq)

            attn_phase(1, qt1, kvt1, kt21, vp1, attn1, gate_hook=gate)
            # tail: jq0/jq1 slices are still resident for the rc1 rows;
            # jq2/jq3 load+project both row blocks
            outproj_chain(1, attn1[1], 1, 0)
            outproj_chain(1, attn1[1], 1, 1)
            for jq in (2, 3):
                outproj_load(jq)
                outproj_chain(1, attn1[0], 0, jq)
                outproj_chain(1, attn1[1], 1, jq)
            flush()

    nc.compile()
    return nc


def _get_nc():
    if "nc" not in _CACHE:
        _CACHE["nc"] = _build()
    return _CACHE["nc"]


def _prep_inputs(hidden_state, w_q, b_q, w_k, b_k, w_v, b_v, w_o, b_o):
    """Host-side sharding/layout prep. Only layout/dtype transforms."""
    xt = np.ascontiguousarray(hidden_state.transpose(0, 2, 1)).astype(NP_BF16)
    # wo pre-arranged to [P, 4, SC, 512] so each column-slice DMA line is
    # one contiguous 16KB run per partition
    wo = np.ascontiguousarray(
        w_o.reshape(SC, P, 4, 512).transpose(1, 2, 0, 3)).astype(NP_BF16)
    bo = np.broadcast_to(b_o.astype(np.float32), (P, HID)).copy()
    in_maps = []
    for g in range(NCORES):
        wq_g = np.ascontiguousarray(
            (w_q[:, g * QF:(g + 1) * QF] * 0.125)
            .reshape(HC, P, QF).transpose(1, 0, 2)).astype(NP_BF16)
        bq_g = np.ascontiguousarray(
            (b_q[g * QF:(g + 1) * QF] * 0.125).reshape(2, P)).astype(np.float32)
        wkv_g = np.ascontiguousarray(np.concatenate(
            [w_k[:, g * D:(g + 1) * D], w_v[:, g * D:(g + 1) * D]],
            axis=1).reshape(HC, P, P).transpose(1, 0, 2)).astype(NP_BF16)
        bkv_g = np.ascontiguousarray(np.concatenate(
            [b_k[g * D:(g + 1) * D], b_v[g * D:(g + 1) * D]])
            .reshape(P, 1)).astype(np.float32)
        in_maps.append({
            "xt": xt, "wq": wq_g, "bq": bq_g, "wkv": wkv_g, "bkv": bkv_g,
            "wo": wo, "bo": bo,
        })
    return in_maps


def kernel(hidden_state, w_q, b_q, w_k, b_k, w_v, b_v, w_o, b_o,
           _trace=False):
    hidden_state = np.asarray(hidden_state, np.float32)
    args = [np.asarray(a, np.float32) for a in
            (w_q, b_q, w_k, b_k, w_v, b_v, w_o, b_o)]
    nc = _get_nc()
    in_maps = _prep_inputs(hidden_state, *args)
    res = bass_utils.run_bass_kernel_spmd(
        nc, in_maps, core_ids=list(range(NCORES)), trace=_trace)
    out = np.concatenate([res.results[g]["out"] for g in range(NCORES)],
                         axis=1).astype(np.float32)
    if _trace:
        _CACHE["last_results"] = res
    return out
